# revision 12
# baseline (speedup 1.0000x reference)
"""BitNet b1.58 ternary-quantized linear on 8 Trainium2 NeuronCores.

Reference computation (single device):
    scale = clip(mean(|W|), 1e-5, 1000)
    q     = ternarize(W / scale, threshold=2/3)  in {-1, 0, +1}
    out   = x @ (q * scale).T + bias             x:[4,2048,4096] W:[4096,4096]

Sharding (2D grid over 8 cores): 4 row-groups of x (M=2048 each) x 2
feature-groups of W (N=2048 each).

Launch A computes the distributed |W| mean (abs on DVE/ACT from a bf16
copy of a 1/8 W row-slice, accumulation entirely in exact f32 PSUM via
ones.T @ |W| column-sum matmuls; the scale shift from the bf16 read is
2.2e-6 relative -> 11 of 16.7M ternary decisions flip). The partials
return to the host, which combines them into the scalar scale and:
  - folds the scale into the x shards (x*s cast to bf16 [K, M] slabs),
    so out = (s*x) @ q.T needs no on-device scaling
  - passes thr = +-(2/3)s as a tiny [128,2] input for the ternarize

Launch B is ordered around two measured hardware behaviors:
  - DMAs effectively execute in EMISSION order (8 semaphore lanes,
    ~320-358 GB/s aggregate; a gated DMA blocks only its lane), so
    everything is emitted in intended arrival order: W n-blocks 0-1
    interleaved into the front of the x stream, W-tail after.
  - The per-engine instruction scheduler hoists DMA-trigger ops ahead
    of compute ops, so a PSUM eviction sharing a queue with quant-gated
    W pushes stalls ~20us at the phase boundary. All DMAs therefore go
    on the sync ring (the 16 SDMA engines serve whichever queue has
    work, so one ring still gets full bandwidth), leaving the ACT queue
    with nothing but the fused Identity+bias evictions.
Phase 1 runs 8 PSUM chains (nb 0-1 x 4 m-chunks) k-outer, consuming
each 1-MiB x pair-slab as it lands; phase 2 runs nb 2-15 as dense
per-(nb,mc) chains (2048 128x128x512 bf16 matmuls total at ~216ns,
the PE floor), with quarter-block W staging/ternarize (DVE is_gt/is_lt
masks -> q bf16) pipelined 2 n-blocks ahead.
"""

import os

import numpy as np
import ml_dtypes

import concourse.bass as bass
import concourse.tile as tile
from concourse import bacc, mybir
from concourse.bass_utils import run_bass_kernel_spmd

N_CORES = 8
R_GRP, F_GRP = 4, 2            # row groups (x) x feature groups (W)
B, S, K = 4, 2048, 4096        # x: [B, S, K]
N_OUT = 4096                   # W: [N_OUT, K]
M_ALL = B * S                  # 8192 rows of x
M_SH = M_ALL // R_GRP          # 2048 rows per core
N_SH = N_OUT // F_GRP          # 2048 out-features per core
WRED = N_OUT // N_CORES        # 512 rows of W per core for the scale reduce
KO = K // 128                  # 32 k-blocks
KP = KO // 2                   # 16 x pair-slabs (2 k-blocks per 1-MiB DMA)
M_CHUNK = 512                  # matmul moving free dim (PSUM bank limit)
N_MC = M_SH // M_CHUNK         # 4 m-chunks
N_NB = N_SH // 128             # 16 n-blocks
QQ = 4                         # k-blocks per W quarter-tile
N_QP = KO // QQ                # 8 quarter-tiles per n-block

THRESH = 2.0 / 3.0
F32 = mybir.dt.float32
BF16 = mybir.dt.bfloat16

_CACHE = {}
LAST_RESULTS = None


def _build_scale():
    """Launch A: partial = sum(|W slice|) via f32-PSUM column-sum matmuls."""
    nc = bacc.Bacc(None, target_bir_lowering=False, num_devices=N_CORES)
    wred_d = nc.dram_tensor("wredb", [WRED, K], BF16, kind="ExternalInput")
    part_d = nc.dram_tensor("partial", [1, 1], F32, kind="ExternalOutput")

    with tile.TileContext(nc) as tc:
        with (
            tc.tile_pool(name="misc", bufs=1) as misc,
            tc.tile_pool(name="redstage", bufs=4) as redstage,
            tc.tile_pool(name="absb", bufs=4) as absb,
            tc.tile_pool(name="psum_s", bufs=1, space="PSUM") as psum_s_pool,
        ):
            ones_bf = misc.tile([128, 1], BF16)
            nc.vector.memset(ones_bf[:], 1.0)
            ps1 = psum_s_pool.tile([1, M_CHUNK], F32)
            wsrc = wred_d.rearrange("(a p) k -> p a k", p=128)
            for t in range(4):
                wf = redstage.tile([128, K], BF16, tag="redstage")
                (nc.sync if t % 2 == 0 else nc.scalar).dma_start(
                    wf[:], wsrc[:, t, :])
                aw = absb.tile([128, K], BF16, tag="absb")
                if t % 2 == 0:
                    # DVE abs: max(w, -w)
                    nw = absb.tile([128, K], BF16, tag="absb")
                    nc.vector.tensor_scalar(
                        nw[:], wf[:], -1.0, None, mybir.AluOpType.mult)
                    nc.vector.tensor_tensor(
                        aw[:], wf[:], nw[:], mybir.AluOpType.max)
                else:
                    nc.scalar.activation(
                        aw[:], wf[:], mybir.ActivationFunctionType.Abs)
                for c in range(K // M_CHUNK):
                    nc.tensor.matmul(
                        ps1[:], lhsT=ones_bf[:],
                        rhs=aw[:, M_CHUNK * c:M_CHUNK * (c + 1)],
                        start=(t == 0 and c == 0),
                        stop=(t == 3 and c == K // M_CHUNK - 1))
            sc = misc.tile([1, 1], F32)
            nc.vector.tensor_reduce(
                sc[:], ps1[:], axis=mybir.AxisListType.X, op=mybir.AluOpType.add)
            nc.sync.dma_start(part_d[:], sc[:])

    nc.compile()
    return nc


def _build_main():
    nc = bacc.Bacc(None, target_bir_lowering=False, num_devices=N_CORES)
    xt_d = nc.dram_tensor("xt_sh", [KP, 128, 2, M_SH], BF16,
                          kind="ExternalInput")
    wt5_d = nc.dram_tensor("wt5", [N_NB, 128, KO, 128], F32, kind="ExternalInput")
    thr_d = nc.dram_tensor("thrs", [128, 2], F32, kind="ExternalInput")
    bias_d = nc.dram_tensor("bias_sh", [N_SH], F32, kind="ExternalInput")
    outT = nc.dram_tensor("outT", [N_SH, M_SH], F32, kind="ExternalOutput")

    with tile.TileContext(nc) as tc:
        with (
            tc.tile_pool(name="misc", bufs=1) as misc,
            tc.tile_pool(name="wq", bufs=10) as wq_pool,
            tc.tile_pool(name="masks", bufs=2) as mask_pool,
            tc.tile_pool(name="qt", bufs=24) as qt_pool,
            tc.tile_pool(name="outp", bufs=4) as out_pool,
            tc.tile_pool(name="psum", bufs=8, space="PSUM") as psum_pool,
        ):
            # emission counter drives engine alternation for every DMA
            _ec = [0]

            def eng():
                return nc.sync

            thrs = misc.tile([128, 2], F32)
            eng().dma_start(thrs[:], thr_d[:, :])
            thr_col, nthr_col = thrs[:, 0:1], thrs[:, 1:2]
            bias_sb = misc.tile([128, N_NB], F32)
            eng().dma_start(bias_sb[:], bias_d.rearrange("(o p) -> p o", p=128))

            warm_w = misc.tile([128, 128], BF16)
            warm_x = misc.tile([128, M_CHUNK], BF16)
            nc.vector.memset(warm_w[:], 0.0)
            nc.vector.memset(warm_x[:], 0.0)

            xt = [None] * KP

            def x_dma(j):
                xj = misc.tile([128, 2, M_SH], BF16, name=f"xt{j}")
                eng().dma_start(xj[:], xt_d[j])
                xt[j] = xj

            def wq_dma(nb, q):
                wq = wq_pool.tile([128, QQ, 128], F32, tag="wq",
                                  name=f"wq{nb}_{q}")
                eng().dma_start(wq[:], wt5_d[nb, :, QQ * q:QQ * (q + 1), :])
                return wq

            # ---- head stream in intended arrival order: W(nb0/nb1) quarter
            # pairs front-loaded into the x pair-slab stream
            # slabs after W quarter-pair q (tuned so each quarter lands
            # ~3us before the PE consumes its first k-block):
            slab_cadence = [0, 1, 1, 2, 2, 2, 2, 2]
            wq_head = {}
            j_it = iter(range(KP))
            for q in range(N_QP):
                wq_head[(0, q)] = wq_dma(0, q)
                wq_head[(1, q)] = wq_dma(1, q)
                for _ in range(slab_cadence[q]):
                    j = next(j_it, None)
                    if j is not None:
                        x_dma(j)
            for j in j_it:
                x_dma(j)

            # ---- ternarize one (nb, quarter): wq f32 -> qt bf16
            qts = {}

            def emit_quant_q(nb, q, wq):
                wq_f = wq[:].rearrange("p a b -> p (a b)")
                mpos = mask_pool.tile([128, QQ * 128], BF16, tag="masks",
                                      name=f"mp{nb}_{q}")
                nc.vector.tensor_scalar(
                    mpos[:], wq_f, thr_col, None, mybir.AluOpType.is_gt)
                mneg = mask_pool.tile([128, QQ * 128], BF16, tag="masks",
                                      name=f"mn{nb}_{q}")
                nc.vector.tensor_scalar(
                    mneg[:], wq_f, nthr_col, None, mybir.AluOpType.is_lt)
                qt = qt_pool.tile([128, QQ, 128], BF16, tag="qt",
                                  name=f"qt{nb}_{q}")
                nc.vector.tensor_tensor(
                    qt[:].rearrange("p a b -> p (a b)"),
                    mpos[:], mneg[:], mybir.AluOpType.subtract)
                qts[(nb, q)] = qt

            for q in range(N_QP):
                for nb in (0, 1):
                    emit_quant_q(nb, q, wq_head[(nb, q)])

            def evict(nb, mc, ps):
                # scale is folded into x on the host; ACT only adds bias.
                # ACT's queue holds nothing but these, so no DMA-hoisting
                # can delay the PSUM drain at the phase boundary.
                ob = out_pool.tile([128, M_CHUNK], F32, tag="outp",
                                   name=f"ob{nb}_{mc}")
                nc.scalar.activation(
                    ob[:], ps[:], mybir.ActivationFunctionType.Identity,
                    bias=bias_sb[:, nb:nb + 1])
                eng().dma_start(
                    outT[128 * nb:128 * (nb + 1),
                         M_CHUNK * mc:M_CHUNK * (mc + 1)], ob[:])

            def mm(ps, nb, mc, ko, start, stop):
                nc.tensor.matmul(
                    ps[:],
                    lhsT=qts[(nb, ko // QQ)][:, ko % QQ, :],
                    rhs=xt[ko // 2][:, ko % 2,
                                    M_CHUNK * mc:M_CHUNK * (mc + 1)],
                    start=start, stop=stop)

            # ---- phase 1: 8 interleaved chains (nb 0-1 x mc 0-3), k-outer.
            # 16 dummy matmuls on memset operands first: they have no DMA
            # dependencies, so the PE runs from ~7us and the HAM clock-gate
            # reaches full rate before the real chains issue (saves the
            # ~6us of 427-630ns cold-ramp matmuls observed at the front).
            ps1 = [psum_pool.tile([128, M_CHUNK], F32, tag="psum",
                                  name=f"ps1_{c}") for c in range(8)]
            for ko in range(KO):
                for c in range(8):
                    nb, mc = divmod(c, 4)
                    if ko == 0 and c == 0:
                        for i in range(42):
                            nc.tensor.matmul(
                                ps1[0][:], lhsT=warm_w[:], rhs=warm_x[:],
                                start=(i == 0), stop=(i == 41),
                                skip_group_check=True)
                    if ko <= 1 and c >= 4:
                        continue      # reordered below: see ko==1 epilogue
                    mm(ps1[c], nb, mc, ko, ko == 0, ko == KO - 1)
                if ko == 1:
                    # nb1's first two k-blocks after nb0's, so the first
                    # real matmuls only gate on qt(0,q0) + the first slab
                    for c in range(4, 8):
                        mm(ps1[c], 1, c - 4, 0, True, False)
                        mm(ps1[c], 1, c - 4, 1, False, False)

            # phase-2 W prep before the phase-1 evictions (emission order
            # is DMA order; the evict DMAs gate on the chain stops)
            def emit_quant(nb):
                for q in range(N_QP):
                    emit_quant_q(nb, q, wq_dma(nb, q))

            emit_quant(2)
            emit_quant(3)
            for c in range(8):
                nb, mc = divmod(c, 4)
                evict(nb, mc, ps1[c])

            # ---- phase 2: n-blocks 2..15 dense, quant pipelined 2 ahead
            for nb in range(2, N_NB):
                if nb + 2 < N_NB:
                    emit_quant(nb + 2)
                for mc in range(N_MC):
                    ps = psum_pool.tile([128, M_CHUNK], F32, tag="psum",
                                        name=f"ps{nb}_{mc}")
                    for ko in range(KO):
                        mm(ps, nb, mc, ko, ko == 0, ko == KO - 1)
                    evict(nb, mc, ps)

    nc.compile()
    return nc


def kernel(x, weight, bias):
    global LAST_RESULTS
    x = np.asarray(x, dtype=np.float32)
    weight = np.ascontiguousarray(np.asarray(weight, dtype=np.float32))
    bias = np.ascontiguousarray(np.asarray(bias, dtype=np.float32))
    if "nc_scale" not in _CACHE:
        _CACHE["nc_scale"] = _build_scale()
        _CACHE["nc_main"] = _build_main()
    nc_scale, nc_main = _CACHE["nc_scale"], _CACHE["nc_main"]

    trace = bool(int(os.environ.get("KERNEL_TRACE", "0")))
    kw = {"trace": True, "trace_cores": [0]} if trace else {}

    # Launch A: distributed |W| partial sums (one distinct 1/8 slice each)
    wb = weight.astype(ml_dtypes.bfloat16)
    in_a = [{"wredb": np.ascontiguousarray(wb[WRED * c:WRED * (c + 1)])}
            for c in range(N_CORES)]
    res_a = run_bass_kernel_spmd(nc_scale, in_a, list(range(N_CORES)), **kw)
    partials = np.array(
        [res_a.results[c]["partial"][0, 0] for c in range(N_CORES)],
        dtype=np.float32)

    # host glue: combine the 8 device partials into scale/threshold
    s = np.float32(np.clip(partials.sum(dtype=np.float32) / (N_OUT * K),
                           1e-5, 1000.0))
    thrs = np.ascontiguousarray(np.broadcast_to(
        np.array([THRESH * s, -THRESH * s], dtype=np.float32), (128, 2)))

    # Launch B: the matmul kernel; scale folded into the x shards
    xr = x.reshape(M_ALL, K)
    in_b = []
    for c in range(N_CORES):
        i, j = c // F_GRP, c % F_GRP
        w_sh = weight[N_SH * j:N_SH * (j + 1)]          # [2048 n, 4096 k]
        # wt5[nb, ki, kb, n] = w_sh[128*nb + n, 128*kb + ki]
        wt5 = np.ascontiguousarray(
            w_sh.reshape(N_NB, 128, KO, 128).transpose(0, 3, 2, 1))
        xT = xr[M_SH * i:M_SH * (i + 1)].T * s          # [K, M] scaled
        # pair-slabs: xt[j, ki, h, m] = xT[256j + 128h + ki, m]
        xts = np.ascontiguousarray(
            xT.reshape(KP, 2, 128, M_SH).transpose(0, 2, 1, 3)
        ).astype(ml_dtypes.bfloat16)
        in_b.append({
            "xt_sh": xts,
            "wt5": wt5,
            "thrs": thrs,
            "bias_sh": bias[N_SH * j:N_SH * (j + 1)],
        })
    res_b = run_bass_kernel_spmd(nc_main, in_b, list(range(N_CORES)), **kw)
    LAST_RESULTS = (res_a, res_b)

    out = np.empty((M_ALL, N_OUT), dtype=np.float32)
    for c in range(N_CORES):
        i, j = c // F_GRP, c % F_GRP
        out[M_SH * i:M_SH * (i + 1), N_SH * j:N_SH * (j + 1)] = \
            res_b.results[c]["outT"].T
    return out.reshape(B, S, N_OUT)


# revision 13
# speedup vs baseline: 1.0000x; 1.0000x over previous
"""BitNet b1.58 ternary-quantized linear on 8 Trainium2 NeuronCores.

Reference computation (single device):
    scale = clip(mean(|W|), 1e-5, 1000)
    q     = ternarize(W / scale, threshold=2/3)  in {-1, 0, +1}
    out   = x @ (q * scale).T + bias             x:[4,2048,4096] W:[4096,4096]

Sharding (2D grid over 8 cores): 4 row-groups of x (M=2048 each) x 2
feature-groups of W (N=2048 each).

Launch A computes the distributed |W| mean (abs on DVE/ACT from a bf16
copy of a 1/8 W row-slice, accumulation entirely in exact f32 PSUM via
ones.T @ |W| column-sum matmuls; the scale shift from the bf16 read is
2.2e-6 relative -> 11 of 16.7M ternary decisions flip). The partials
return to the host, which combines them into the scalar scale and:
  - folds the scale into the x shards (x*s cast to bf16 [K, M] slabs),
    so out = (s*x) @ q.T needs no on-device scaling
  - passes thr = +-(2/3)s as a tiny [128,2] input for the ternarize

Launch B is ordered around two measured hardware behaviors:
  - DMAs effectively execute in EMISSION order (8 semaphore lanes,
    ~320-358 GB/s aggregate; a gated DMA blocks only its lane), so
    everything is emitted in intended arrival order: W n-blocks 0-1
    interleaved into the front of the x stream, W-tail after.
  - The per-engine instruction scheduler hoists DMA-trigger ops ahead
    of compute ops, so a PSUM eviction sharing a queue with quant-gated
    W pushes stalls ~20us at the phase boundary. All DMAs therefore go
    on the sync ring (the 16 SDMA engines serve whichever queue has
    work, so one ring still gets full bandwidth), leaving the ACT queue
    with nothing but the fused Identity+bias evictions.
Phase 1 runs 8 PSUM chains (nb 0-1 x 4 m-chunks) k-outer, consuming
each 1-MiB x pair-slab as it lands; phase 2 runs nb 2-15 as dense
per-(nb,mc) chains (2048 128x128x512 bf16 matmuls total at ~216ns,
the PE floor), with quarter-block W staging/ternarize (DVE is_gt/is_lt
masks -> q bf16) pipelined 2 n-blocks ahead.
"""

import os

import numpy as np
import ml_dtypes

import concourse.bass as bass
import concourse.tile as tile
from concourse import bacc, mybir
from concourse.bass_utils import run_bass_kernel_spmd

N_CORES = 8
R_GRP, F_GRP = 4, 2            # row groups (x) x feature groups (W)
B, S, K = 4, 2048, 4096        # x: [B, S, K]
N_OUT = 4096                   # W: [N_OUT, K]
M_ALL = B * S                  # 8192 rows of x
M_SH = M_ALL // R_GRP          # 2048 rows per core
N_SH = N_OUT // F_GRP          # 2048 out-features per core
WRED = N_OUT // N_CORES        # 512 rows of W per core for the scale reduce
KO = K // 128                  # 32 k-blocks
KP = KO // 2                   # 16 x pair-slabs (2 k-blocks per 1-MiB DMA)
M_CHUNK = 512                  # matmul moving free dim (PSUM bank limit)
N_MC = M_SH // M_CHUNK         # 4 m-chunks
N_NB = N_SH // 128             # 16 n-blocks
QQ = 4                         # k-blocks per W quarter-tile
N_QP = KO // QQ                # 8 quarter-tiles per n-block

THRESH = 2.0 / 3.0
F32 = mybir.dt.float32
BF16 = mybir.dt.bfloat16

_CACHE = {}
LAST_RESULTS = None


def _build_scale():
    """Launch A: partial = sum(|W slice|) via f32-PSUM column-sum matmuls."""
    nc = bacc.Bacc(None, target_bir_lowering=False, num_devices=N_CORES)
    wred_d = nc.dram_tensor("wredb", [WRED, K], BF16, kind="ExternalInput")
    part_d = nc.dram_tensor("partial", [1, 1], F32, kind="ExternalOutput")

    with tile.TileContext(nc) as tc:
        with (
            tc.tile_pool(name="misc", bufs=1) as misc,
            tc.tile_pool(name="redstage", bufs=4) as redstage,
            tc.tile_pool(name="absb", bufs=4) as absb,
            tc.tile_pool(name="psum_s", bufs=1, space="PSUM") as psum_s_pool,
        ):
            ones_bf = misc.tile([128, 1], BF16)
            nc.vector.memset(ones_bf[:], 1.0)
            ps1 = psum_s_pool.tile([1, M_CHUNK], F32)
            wsrc = wred_d.rearrange("(a p) k -> p a k", p=128)
            for t in range(4):
                wf = redstage.tile([128, K], BF16, tag="redstage")
                (nc.sync if t % 2 == 0 else nc.scalar).dma_start(
                    wf[:], wsrc[:, t, :])
                aw = absb.tile([128, K], BF16, tag="absb")
                if t % 2 == 0:
                    # DVE abs: max(w, -w)
                    nw = absb.tile([128, K], BF16, tag="absb")
                    nc.vector.tensor_scalar(
                        nw[:], wf[:], -1.0, None, mybir.AluOpType.mult)
                    nc.vector.tensor_tensor(
                        aw[:], wf[:], nw[:], mybir.AluOpType.max)
                else:
                    nc.scalar.activation(
                        aw[:], wf[:], mybir.ActivationFunctionType.Abs)
                for c in range(K // M_CHUNK):
                    nc.tensor.matmul(
                        ps1[:], lhsT=ones_bf[:],
                        rhs=aw[:, M_CHUNK * c:M_CHUNK * (c + 1)],
                        start=(t == 0 and c == 0),
                        stop=(t == 3 and c == K // M_CHUNK - 1))
            sc = misc.tile([1, 1], F32)
            nc.vector.tensor_reduce(
                sc[:], ps1[:], axis=mybir.AxisListType.X, op=mybir.AluOpType.add)
            nc.sync.dma_start(part_d[:], sc[:])

    nc.compile()
    return nc


def _build_main():
    nc = bacc.Bacc(None, target_bir_lowering=False, num_devices=N_CORES)
    xt_d = nc.dram_tensor("xt_sh", [KP, 128, 2, M_SH], BF16,
                          kind="ExternalInput")
    wt5_d = nc.dram_tensor("wt5", [N_NB, 128, KO, 128], F32, kind="ExternalInput")
    thr_d = nc.dram_tensor("thrs", [128, 2], F32, kind="ExternalInput")
    bias_d = nc.dram_tensor("bias_sh", [N_SH], F32, kind="ExternalInput")
    outT = nc.dram_tensor("outT", [N_SH, M_SH], F32, kind="ExternalOutput")

    with tile.TileContext(nc) as tc:
        with (
            tc.tile_pool(name="misc", bufs=1) as misc,
            tc.tile_pool(name="wq", bufs=10) as wq_pool,
            tc.tile_pool(name="masks", bufs=2) as mask_pool,
            tc.tile_pool(name="qt", bufs=24) as qt_pool,
            tc.tile_pool(name="outp", bufs=4) as out_pool,
            tc.tile_pool(name="psum", bufs=8, space="PSUM") as psum_pool,
        ):
            # emission counter drives engine alternation for every DMA
            _ec = [0]

            def eng():
                return nc.sync

            thrs = misc.tile([128, 2], F32)
            eng().dma_start(thrs[:], thr_d[:, :])
            thr_col, nthr_col = thrs[:, 0:1], thrs[:, 1:2]
            bias_sb = misc.tile([128, N_NB], F32)
            eng().dma_start(bias_sb[:], bias_d.rearrange("(o p) -> p o", p=128))

            xt = [None] * KP

            def x_dma(j):
                xj = misc.tile([128, 2, M_SH], BF16, name=f"xt{j}")
                eng().dma_start(xj[:], xt_d[j])
                xt[j] = xj

            def wq_dma(nb, q):
                wq = wq_pool.tile([128, QQ, 128], F32, tag="wq",
                                  name=f"wq{nb}_{q}")
                eng().dma_start(wq[:], wt5_d[nb, :, QQ * q:QQ * (q + 1), :])
                return wq

            # ---- head stream in intended arrival order: W(nb0/nb1) quarter
            # pairs front-loaded into the x pair-slab stream
            # slabs after W quarter-pair q (tuned so each quarter lands
            # ~3us before the PE consumes its first k-block):
            slab_cadence = [0, 1, 1, 2, 2, 2, 2, 2]
            wq_head = {}
            j_it = iter(range(KP))
            for q in range(N_QP):
                wq_head[(0, q)] = wq_dma(0, q)
                wq_head[(1, q)] = wq_dma(1, q)
                for _ in range(slab_cadence[q]):
                    j = next(j_it, None)
                    if j is not None:
                        x_dma(j)
            for j in j_it:
                x_dma(j)

            # ---- ternarize one (nb, quarter): wq f32 -> qt bf16
            qts = {}

            def emit_quant_q(nb, q, wq):
                wq_f = wq[:].rearrange("p a b -> p (a b)")
                mpos = mask_pool.tile([128, QQ * 128], BF16, tag="masks",
                                      name=f"mp{nb}_{q}")
                nc.vector.tensor_scalar(
                    mpos[:], wq_f, thr_col, None, mybir.AluOpType.is_gt)
                mneg = mask_pool.tile([128, QQ * 128], BF16, tag="masks",
                                      name=f"mn{nb}_{q}")
                nc.vector.tensor_scalar(
                    mneg[:], wq_f, nthr_col, None, mybir.AluOpType.is_lt)
                qt = qt_pool.tile([128, QQ, 128], BF16, tag="qt",
                                  name=f"qt{nb}_{q}")
                nc.vector.tensor_tensor(
                    qt[:].rearrange("p a b -> p (a b)"),
                    mpos[:], mneg[:], mybir.AluOpType.subtract)
                qts[(nb, q)] = qt

            for q in range(N_QP):
                for nb in (0, 1):
                    emit_quant_q(nb, q, wq_head[(nb, q)])

            def evict(nb, mc, ps):
                # scale is folded into x on the host; ACT only adds bias.
                # ACT's queue holds nothing but these, so no DMA-hoisting
                # can delay the PSUM drain at the phase boundary.
                ob = out_pool.tile([128, M_CHUNK], F32, tag="outp",
                                   name=f"ob{nb}_{mc}")
                nc.scalar.activation(
                    ob[:], ps[:], mybir.ActivationFunctionType.Identity,
                    bias=bias_sb[:, nb:nb + 1])
                eng().dma_start(
                    outT[128 * nb:128 * (nb + 1),
                         M_CHUNK * mc:M_CHUNK * (mc + 1)], ob[:])

            def mm(ps, nb, mc, ko, start, stop):
                nc.tensor.matmul(
                    ps[:],
                    lhsT=qts[(nb, ko // QQ)][:, ko % QQ, :],
                    rhs=xt[ko // 2][:, ko % 2,
                                    M_CHUNK * mc:M_CHUNK * (mc + 1)],
                    start=start, stop=stop)

            # ---- phase 1: 8 interleaved chains (nb 0-1 x mc 0-3), k-outer
            ps1 = [psum_pool.tile([128, M_CHUNK], F32, tag="psum",
                                  name=f"ps1_{c}") for c in range(8)]
            for ko in range(KO):
                for c in range(8):
                    nb, mc = divmod(c, 4)
                    mm(ps1[c], nb, mc, ko, ko == 0, ko == KO - 1)

            # phase-2 W prep before the phase-1 evictions (emission order
            # is DMA order; the evict DMAs gate on the chain stops)
            def emit_quant(nb):
                for q in range(N_QP):
                    emit_quant_q(nb, q, wq_dma(nb, q))

            emit_quant(2)
            emit_quant(3)
            for c in range(8):
                nb, mc = divmod(c, 4)
                evict(nb, mc, ps1[c])

            # ---- phase 2: n-blocks 2..15 dense, quant pipelined 2 ahead
            for nb in range(2, N_NB):
                if nb + 2 < N_NB:
                    emit_quant(nb + 2)
                for mc in range(N_MC):
                    ps = psum_pool.tile([128, M_CHUNK], F32, tag="psum",
                                        name=f"ps{nb}_{mc}")
                    for ko in range(KO):
                        mm(ps, nb, mc, ko, ko == 0, ko == KO - 1)
                    evict(nb, mc, ps)

    nc.compile()
    return nc


def kernel(x, weight, bias):
    global LAST_RESULTS
    x = np.asarray(x, dtype=np.float32)
    weight = np.ascontiguousarray(np.asarray(weight, dtype=np.float32))
    bias = np.ascontiguousarray(np.asarray(bias, dtype=np.float32))
    if "nc_scale" not in _CACHE:
        _CACHE["nc_scale"] = _build_scale()
        _CACHE["nc_main"] = _build_main()
    nc_scale, nc_main = _CACHE["nc_scale"], _CACHE["nc_main"]

    trace = bool(int(os.environ.get("KERNEL_TRACE", "0")))
    kw = {"trace": True, "trace_cores": [0]} if trace else {}

    # Launch A: distributed |W| partial sums (one distinct 1/8 slice each)
    wb = weight.astype(ml_dtypes.bfloat16)
    in_a = [{"wredb": np.ascontiguousarray(wb[WRED * c:WRED * (c + 1)])}
            for c in range(N_CORES)]
    res_a = run_bass_kernel_spmd(nc_scale, in_a, list(range(N_CORES)), **kw)
    partials = np.array(
        [res_a.results[c]["partial"][0, 0] for c in range(N_CORES)],
        dtype=np.float32)

    # host glue: combine the 8 device partials into scale/threshold
    s = np.float32(np.clip(partials.sum(dtype=np.float32) / (N_OUT * K),
                           1e-5, 1000.0))
    thrs = np.ascontiguousarray(np.broadcast_to(
        np.array([THRESH * s, -THRESH * s], dtype=np.float32), (128, 2)))

    # Launch B: the matmul kernel; scale folded into the x shards
    xr = x.reshape(M_ALL, K)
    in_b = []
    for c in range(N_CORES):
        i, j = c // F_GRP, c % F_GRP
        w_sh = weight[N_SH * j:N_SH * (j + 1)]          # [2048 n, 4096 k]
        # wt5[nb, ki, kb, n] = w_sh[128*nb + n, 128*kb + ki]
        wt5 = np.ascontiguousarray(
            w_sh.reshape(N_NB, 128, KO, 128).transpose(0, 3, 2, 1))
        xT = xr[M_SH * i:M_SH * (i + 1)].T * s          # [K, M] scaled
        # pair-slabs: xt[j, ki, h, m] = xT[256j + 128h + ki, m]
        xts = np.ascontiguousarray(
            xT.reshape(KP, 2, 128, M_SH).transpose(0, 2, 1, 3)
        ).astype(ml_dtypes.bfloat16)
        in_b.append({
            "xt_sh": xts,
            "wt5": wt5,
            "thrs": thrs,
            "bias_sh": bias[N_SH * j:N_SH * (j + 1)],
        })
    res_b = run_bass_kernel_spmd(nc_main, in_b, list(range(N_CORES)), **kw)
    LAST_RESULTS = (res_a, res_b)

    out = np.empty((M_ALL, N_OUT), dtype=np.float32)
    for c in range(N_CORES):
        i, j = c // F_GRP, c % F_GRP
        out[M_SH * i:M_SH * (i + 1), N_SH * j:N_SH * (j + 1)] = \
            res_b.results[c]["outT"].T
    return out.reshape(B, S, N_OUT)


# revision 14
# speedup vs baseline: 1.0192x; 1.0191x over previous
"""BitNet b1.58 ternary-quantized linear on 8 Trainium2 NeuronCores.

Reference computation (single device):
    scale = clip(mean(|W|), 1e-5, 1000)
    q     = ternarize(W / scale, threshold=2/3)  in {-1, 0, +1}
    out   = x @ (q * scale).T + bias             x:[4,2048,4096] W:[4096,4096]

Sharding (2D grid over 8 cores): 4 row-groups of x (M=2048 each) x 2
feature-groups of W (N=2048 each).

Launch A computes the distributed |W| mean (abs on DVE/ACT from a bf16
copy of a 1/8 W row-slice, accumulation entirely in exact f32 PSUM via
ones.T @ |W| column-sum matmuls; the scale shift from the bf16 read is
2.2e-6 relative -> 11 of 16.7M ternary decisions flip). The partials
return to the host, which combines them into the scalar scale and:
  - folds the scale into the x shards (x*s cast to bf16 [K, M] slabs),
    so out = (s*x) @ q.T needs no on-device scaling
  - passes thr = +-(2/3)s as a tiny [128,2] input for the ternarize

Launch B is ordered around two measured hardware behaviors:
  - DMAs effectively execute in EMISSION order (8 semaphore lanes,
    ~320-358 GB/s aggregate; a gated DMA blocks only its lane), so
    everything is emitted in intended arrival order: W n-blocks 0-1
    interleaved into the front of the x stream, W-tail after.
  - The per-engine instruction scheduler hoists DMA-trigger ops ahead
    of compute ops, so a PSUM eviction sharing a queue with quant-gated
    W pushes stalls ~20us at the phase boundary. All DMAs therefore go
    on the sync ring (the 16 SDMA engines serve whichever queue has
    work, so one ring still gets full bandwidth), leaving the ACT queue
    with nothing but the fused Identity+bias evictions.
Phase 1 runs 8 PSUM chains (nb 0-1 x 4 m-chunks) k-outer, consuming
each 1-MiB x pair-slab as it lands; phase 2 runs nb 2-15 as dense
per-(nb,mc) chains (2048 128x128x512 bf16 matmuls total at ~216ns,
the PE floor), with quarter-block W staging/ternarize (DVE is_gt/is_lt
masks -> q bf16) pipelined 2 n-blocks ahead.
"""

import os

import numpy as np
import ml_dtypes

import concourse.bass as bass
import concourse.tile as tile
from concourse import bacc, mybir
from concourse.bass_utils import run_bass_kernel_spmd

N_CORES = 8
R_GRP, F_GRP = 4, 2            # row groups (x) x feature groups (W)
B, S, K = 4, 2048, 4096        # x: [B, S, K]
N_OUT = 4096                   # W: [N_OUT, K]
M_ALL = B * S                  # 8192 rows of x
M_SH = M_ALL // R_GRP          # 2048 rows per core
N_SH = N_OUT // F_GRP          # 2048 out-features per core
WRED = N_OUT // N_CORES        # 512 rows of W per core for the scale reduce
KO = K // 128                  # 32 k-blocks
KP = KO // 2                   # 16 x pair-slabs (2 k-blocks per 1-MiB DMA)
M_CHUNK = 512                  # matmul moving free dim (PSUM bank limit)
N_MC = M_SH // M_CHUNK         # 4 m-chunks
N_NB = N_SH // 128             # 16 n-blocks
QQ = 4                         # k-blocks per W quarter-tile
N_QP = KO // QQ                # 8 quarter-tiles per n-block

THRESH = 2.0 / 3.0
F32 = mybir.dt.float32
BF16 = mybir.dt.bfloat16

_CACHE = {}
LAST_RESULTS = None


def _build_scale():
    """Launch A: partial = sum(|W slice|) via f32-PSUM column-sum matmuls."""
    nc = bacc.Bacc(None, target_bir_lowering=False, num_devices=N_CORES)
    wred_d = nc.dram_tensor("wredb", [WRED, K], BF16, kind="ExternalInput")
    part_d = nc.dram_tensor("partial", [1, 1], F32, kind="ExternalOutput")

    with tile.TileContext(nc) as tc:
        with (
            tc.tile_pool(name="misc", bufs=1) as misc,
            tc.tile_pool(name="redstage", bufs=4) as redstage,
            tc.tile_pool(name="absb", bufs=4) as absb,
            tc.tile_pool(name="psum_s", bufs=1, space="PSUM") as psum_s_pool,
        ):
            ones_bf = misc.tile([128, 1], BF16)
            nc.vector.memset(ones_bf[:], 1.0)
            ps1 = psum_s_pool.tile([1, M_CHUNK], F32)
            wsrc = wred_d.rearrange("(a p) k -> p a k", p=128)
            for t in range(4):
                wf = redstage.tile([128, K], BF16, tag="redstage")
                (nc.sync if t % 2 == 0 else nc.scalar).dma_start(
                    wf[:], wsrc[:, t, :])
                aw = absb.tile([128, K], BF16, tag="absb")
                if t % 2 == 0:
                    # DVE abs: max(w, -w)
                    nw = absb.tile([128, K], BF16, tag="absb")
                    nc.vector.tensor_scalar(
                        nw[:], wf[:], -1.0, None, mybir.AluOpType.mult)
                    nc.vector.tensor_tensor(
                        aw[:], wf[:], nw[:], mybir.AluOpType.max)
                else:
                    nc.scalar.activation(
                        aw[:], wf[:], mybir.ActivationFunctionType.Abs)
                for c in range(K // M_CHUNK):
                    nc.tensor.matmul(
                        ps1[:], lhsT=ones_bf[:],
                        rhs=aw[:, M_CHUNK * c:M_CHUNK * (c + 1)],
                        start=(t == 0 and c == 0),
                        stop=(t == 3 and c == K // M_CHUNK - 1))
            sc = misc.tile([1, 1], F32)
            nc.vector.tensor_reduce(
                sc[:], ps1[:], axis=mybir.AxisListType.X, op=mybir.AluOpType.add)
            nc.sync.dma_start(part_d[:], sc[:])

    nc.compile()
    return nc


def _build_main():
    nc = bacc.Bacc(None, target_bir_lowering=False, num_devices=N_CORES)
    xt_d = nc.dram_tensor("xt_sh", [KP, 128, 2, M_SH], BF16,
                          kind="ExternalInput")
    wt5_d = nc.dram_tensor("wt5", [N_NB, 128, KO, 128], F32, kind="ExternalInput")
    # packed head: [thr, nthr | bias cols | W(0,q0) | W(1,q0)] in one DMA
    HW0, HW1 = 2 + N_NB, 2 + N_NB + QQ * 128
    head_d = nc.dram_tensor("head_in", [128, 2 + N_NB + 2 * QQ * 128], F32,
                            kind="ExternalInput")
    outT = nc.dram_tensor("outT", [N_SH, M_SH], F32, kind="ExternalOutput")

    with tile.TileContext(nc) as tc:
        with (
            tc.tile_pool(name="misc", bufs=1) as misc,
            tc.tile_pool(name="wq", bufs=10) as wq_pool,
            tc.tile_pool(name="masks", bufs=2) as mask_pool,
            tc.tile_pool(name="qt", bufs=24) as qt_pool,
            tc.tile_pool(name="outp", bufs=4) as out_pool,
            tc.tile_pool(name="psum", bufs=8, space="PSUM") as psum_pool,
        ):
            # emission counter drives engine alternation for every DMA
            _ec = [0]

            def eng():
                return nc.sync

            head = misc.tile([128, 2 + N_NB + 2 * QQ * 128], F32)
            eng().dma_start(head[:], head_d[:, :])
            thr_col, nthr_col = head[:, 0:1], head[:, 1:2]
            bias_sb = head[:, 2:2 + N_NB]

            xt = [None] * KP

            def x_dma(j):
                xj = misc.tile([128, 2, M_SH], BF16, name=f"xt{j}")
                eng().dma_start(xj[:], xt_d[j])
                xt[j] = xj

            def wq_dma(nb, q):
                wq = wq_pool.tile([128, QQ, 128], F32, tag="wq",
                                  name=f"wq{nb}_{q}")
                eng().dma_start(wq[:], wt5_d[nb, :, QQ * q:QQ * (q + 1), :])
                return wq

            # ---- head stream in intended arrival order: W(nb0/nb1) quarter
            # pairs front-loaded into the x pair-slab stream
            # slabs after W quarter-pair q (tuned so each quarter lands
            # ~3us before the PE consumes its first k-block):
            slab_cadence = [1, 1, 2, 2, 2, 2, 2]
            wq_head = {}
            j_it = iter(range(KP))
            x_dma(next(j_it))
            for q in range(1, N_QP):
                wq_head[(0, q)] = wq_dma(0, q)
                wq_head[(1, q)] = wq_dma(1, q)
                for _ in range(slab_cadence[q - 1]):
                    j = next(j_it, None)
                    if j is not None:
                        x_dma(j)
            for j in j_it:
                x_dma(j)

            # ---- ternarize one (nb, quarter): wq f32 -> qt bf16
            qts = {}

            def emit_quant_q(nb, q, wq):
                wq_f = wq[:].rearrange("p a b -> p (a b)")
                mpos = mask_pool.tile([128, QQ * 128], BF16, tag="masks",
                                      name=f"mp{nb}_{q}")
                nc.vector.tensor_scalar(
                    mpos[:], wq_f, thr_col, None, mybir.AluOpType.is_gt)
                mneg = mask_pool.tile([128, QQ * 128], BF16, tag="masks",
                                      name=f"mn{nb}_{q}")
                nc.vector.tensor_scalar(
                    mneg[:], wq_f, nthr_col, None, mybir.AluOpType.is_lt)
                qt = qt_pool.tile([128, QQ, 128], BF16, tag="qt",
                                  name=f"qt{nb}_{q}")
                nc.vector.tensor_tensor(
                    qt[:].rearrange("p a b -> p (a b)"),
                    mpos[:], mneg[:], mybir.AluOpType.subtract)
                qts[(nb, q)] = qt

            def quant_head0(nb):
                lo = HW0 if nb == 0 else HW1
                class _W:
                    pass
                wq_f = head[:, lo:lo + QQ * 128]
                mpos = mask_pool.tile([128, QQ * 128], BF16, tag="masks",
                                      name=f"mp{nb}_0")
                nc.vector.tensor_scalar(
                    mpos[:], wq_f, thr_col, None, mybir.AluOpType.is_gt)
                mneg = mask_pool.tile([128, QQ * 128], BF16, tag="masks",
                                      name=f"mn{nb}_0")
                nc.vector.tensor_scalar(
                    mneg[:], wq_f, nthr_col, None, mybir.AluOpType.is_lt)
                qt = qt_pool.tile([128, QQ, 128], BF16, tag="qt",
                                  name=f"qt{nb}_0")
                nc.vector.tensor_tensor(
                    qt[:].rearrange("p a b -> p (a b)"),
                    mpos[:], mneg[:], mybir.AluOpType.subtract)
                qts[(nb, 0)] = qt

            quant_head0(0)
            quant_head0(1)
            for q in range(1, N_QP):
                for nb in (0, 1):
                    emit_quant_q(nb, q, wq_head[(nb, q)])

            def evict(nb, mc, ps):
                # scale is folded into x on the host; ACT only adds bias.
                # ACT's queue holds nothing but these, so no DMA-hoisting
                # can delay the PSUM drain at the phase boundary.
                ob = out_pool.tile([128, M_CHUNK], F32, tag="outp",
                                   name=f"ob{nb}_{mc}")
                nc.scalar.activation(
                    ob[:], ps[:], mybir.ActivationFunctionType.Identity,
                    bias=bias_sb[:, nb:nb + 1])
                eng().dma_start(
                    outT[128 * nb:128 * (nb + 1),
                         M_CHUNK * mc:M_CHUNK * (mc + 1)], ob[:])

            def mm(ps, nb, mc, ko, start, stop):
                nc.tensor.matmul(
                    ps[:],
                    lhsT=qts[(nb, ko // QQ)][:, ko % QQ, :],
                    rhs=xt[ko // 2][:, ko % 2,
                                    M_CHUNK * mc:M_CHUNK * (mc + 1)],
                    start=start, stop=stop)

            # ---- phase 1: 8 interleaved chains (nb 0-1 x mc 0-3), k-outer
            ps1 = [psum_pool.tile([128, M_CHUNK], F32, tag="psum",
                                  name=f"ps1_{c}") for c in range(8)]
            for ko in range(KO):
                for c in range(8):
                    nb, mc = divmod(c, 4)
                    mm(ps1[c], nb, mc, ko, ko == 0, ko == KO - 1)

            # phase-2 W prep before the phase-1 evictions (emission order
            # is DMA order; the evict DMAs gate on the chain stops)
            def emit_quant(nb):
                for q in range(N_QP):
                    emit_quant_q(nb, q, wq_dma(nb, q))

            emit_quant(2)
            emit_quant(3)
            for c in range(8):
                nb, mc = divmod(c, 4)
                evict(nb, mc, ps1[c])

            # ---- phase 2: n-blocks 2..15 dense, quant pipelined 2 ahead
            for nb in range(2, N_NB):
                if nb + 2 < N_NB:
                    emit_quant(nb + 2)
                for mc in range(N_MC):
                    ps = psum_pool.tile([128, M_CHUNK], F32, tag="psum",
                                        name=f"ps{nb}_{mc}")
                    for ko in range(KO):
                        mm(ps, nb, mc, ko, ko == 0, ko == KO - 1)
                    evict(nb, mc, ps)

    nc.compile()
    return nc


def kernel(x, weight, bias):
    global LAST_RESULTS
    x = np.asarray(x, dtype=np.float32)
    weight = np.ascontiguousarray(np.asarray(weight, dtype=np.float32))
    bias = np.ascontiguousarray(np.asarray(bias, dtype=np.float32))
    if "nc_scale" not in _CACHE:
        _CACHE["nc_scale"] = _build_scale()
        _CACHE["nc_main"] = _build_main()
    nc_scale, nc_main = _CACHE["nc_scale"], _CACHE["nc_main"]

    trace = bool(int(os.environ.get("KERNEL_TRACE", "0")))
    kw = {"trace": True, "trace_cores": [0]} if trace else {}

    # Launch A: distributed |W| partial sums (one distinct 1/8 slice each)
    wb = weight.astype(ml_dtypes.bfloat16)
    in_a = [{"wredb": np.ascontiguousarray(wb[WRED * c:WRED * (c + 1)])}
            for c in range(N_CORES)]
    res_a = run_bass_kernel_spmd(nc_scale, in_a, list(range(N_CORES)), **kw)
    partials = np.array(
        [res_a.results[c]["partial"][0, 0] for c in range(N_CORES)],
        dtype=np.float32)

    # host glue: combine the 8 device partials into scale/threshold
    s = np.float32(np.clip(partials.sum(dtype=np.float32) / (N_OUT * K),
                           1e-5, 1000.0))
    thrs = np.broadcast_to(
        np.array([THRESH * s, -THRESH * s], dtype=np.float32), (128, 2))

    # Launch B: the matmul kernel; scale folded into the x shards
    xr = x.reshape(M_ALL, K)
    in_b = []
    for c in range(N_CORES):
        i, j = c // F_GRP, c % F_GRP
        w_sh = weight[N_SH * j:N_SH * (j + 1)]          # [2048 n, 4096 k]
        # wt5[nb, ki, kb, n] = w_sh[128*nb + n, 128*kb + ki]
        wt5 = np.ascontiguousarray(
            w_sh.reshape(N_NB, 128, KO, 128).transpose(0, 3, 2, 1))
        xT = xr[M_SH * i:M_SH * (i + 1)].T * s          # [K, M] scaled
        # pair-slabs: xt[j, ki, h, m] = xT[256j + 128h + ki, m]
        xts = np.ascontiguousarray(
            xT.reshape(KP, 2, 128, M_SH).transpose(0, 2, 1, 3)
        ).astype(ml_dtypes.bfloat16)
        bias_cols = bias[N_SH * j:N_SH * (j + 1)].reshape(N_NB, 128).T
        head_in = np.ascontiguousarray(np.concatenate(
            [thrs, bias_cols,
             wt5[0, :, :QQ, :].reshape(128, QQ * 128),
             wt5[1, :, :QQ, :].reshape(128, QQ * 128)], axis=1,
            dtype=np.float32))
        in_b.append({
            "xt_sh": xts,
            "wt5": wt5,
            "head_in": head_in,
        })
    res_b = run_bass_kernel_spmd(nc_main, in_b, list(range(N_CORES)), **kw)
    LAST_RESULTS = (res_a, res_b)

    out = np.empty((M_ALL, N_OUT), dtype=np.float32)
    for c in range(N_CORES):
        i, j = c // F_GRP, c % F_GRP
        out[M_SH * i:M_SH * (i + 1), N_SH * j:N_SH * (j + 1)] = \
            res_b.results[c]["outT"].T
    return out.reshape(B, S, N_OUT)
